# revision 1
# baseline (speedup 1.0000x reference)
"""Trainium2 distributed kernel for nn_AutoCorrelationLayer (FourierBlock).

Only 32 of 1025 rfft bins survive, so both FFTs collapse to small DFT
matmuls and Wq/Wo fold into per-mode weights on the host (stored as r/i/-i
blocks so the complex mix needs no on-device negation).  Pipeline per core:
stage A (DFT, hidden under input DMA) -> A2A #1 (batch->mode) -> stage B
(per-mode mix, free dim 512, col-packed) -> A2A #2 (mode->batch) -> stage C
(iDFT, row-packed) -> stores.

Trace-driven scheduling decisions:
  - ncfw collectives have a ~50-70us init anchored to NEFF exec start; the
    first mesh cannot begin before ~70-82us regardless of trigger time, so
    A2A #1 is the FIRST collective (no warm-up dummy) and the whole front
    end (8MB input, 6MB weights, stage A) hides under that wall for free.
  - Stage C is bound by PSUM->SBUF evictions (only ACT/DVE have PSUM
    ports) and the 8MB output store.  Evictions run in bank-pairs split
    17/15 toward ACT (DVE pair-copies measure ~10% slower), into a full
    4-batch SBUF buffer (no store backpressure); stores stream on both
    HWDGE rings with contiguous per-partition layout (host un-permutes).
  - X2 triggers one staging-DMA early (the 4th completes inside the mesh
    arrival barrier); zero-bias inputs skip the mode-0 bias-add chain.
All matmuls bf16 with f32 PSUM accumulation; rel err ~3.9e-3 (gate 2e-2).
"""

import sys
from contextlib import ExitStack

import numpy as np

sys.path.insert(0, "/opt/trn_rl_repo")

import concourse.bass as bass  # noqa: E402
import concourse.mybir as mybir  # noqa: E402
from concourse.bass_utils import run_bass_kernel_spmd  # noqa: E402

import ml_dtypes  # noqa: E402

BF16 = ml_dtypes.bfloat16

B, L, E, MODES = 32, 2048, 512, 32
NCORES = 8
BL = B // NCORES          # local batches per core (4)
ML = MODES // NCORES      # local modes per core (4)
NCH = E // 128            # 128-partition chunks of E (4)
KT = L // 128             # k-tiles along L (16)
GRP = 2 * ML              # cols per mode-group in DFT output (4 cos + 4 sin)

_nc_cache = {}


def build_nc(with_bias=True):
    f32 = mybir.dt.float32
    bf16 = mybir.dt.bfloat16

    nc = bass.Bass()

    q_ext = nc.declare_dram_parameter("q", [BL, L, E], bf16, isOutput=False)
    ft_ext = nc.declare_dram_parameter("ft", [128, KT * 64], bf16, isOutput=False)
    w_ext = nc.declare_dram_parameter("w", [ML, 128, 3 * NCH * E], bf16, isOutput=False)
    g_ext = nc.declare_dram_parameter("g", [128, L], bf16, isOutput=False)
    mb_ext = nc.declare_dram_parameter("mb", [B, E], f32, isOutput=False)
    out_ext = nc.declare_dram_parameter("out", [BL, 128, KT * E], bf16, isOutput=True)

    # A2A bounces. a1: [dest j][b 4][i 512][col 8]; a2: [dest j][b 4][tr 8][p 512]
    a1_in = nc.dram_tensor("a1_in", [NCORES, BL * E * GRP], bf16)
    a1_out = nc.dram_tensor("a1_out", [NCORES, BL * E * GRP], bf16)
    a2_in = nc.dram_tensor("a2_in", [NCORES, BL * GRP * E], bf16)
    a2_out = nc.dram_tensor("a2_out", [NCORES, BL * GRP * E], bf16)
    rg = [list(range(NCORES))]

    with ExitStack() as ctx:
        ft_sb = ctx.enter_context(nc.sbuf_tensor([128, KT * 64], bf16))
        w_sb = ctx.enter_context(nc.sbuf_tensor([128, ML * 3 * NCH * E], bf16))
        g_sb = ctx.enter_context(nc.sbuf_tensor([128, L], bf16))
        mb_sb = ctx.enter_context(nc.sbuf_tensor([B, E], f32))
        qk_sb = ctx.enter_context(nc.sbuf_tensor([128, 2 * KT * E], bf16))
        qa_sb = ctx.enter_context(nc.sbuf_tensor([128, BL * NCH * 64], bf16))
        qm_sb = ctx.enter_context(nc.sbuf_tensor([128, NCH * B * GRP], bf16))
        ys_sb = ctx.enter_context(nc.sbuf_tensor([128, 2 * E], bf16))
        yst_sb = ctx.enter_context(nc.sbuf_tensor([128, BL * E], bf16))
        ob_sb = ctx.enter_context(nc.sbuf_tensor([128, BL * 16 * E], bf16))
        ps = ctx.enter_context(nc.psum_tensor([128, 4096], f32))
        (sFt, sMb, sW, sG, sMA, sEA, sS1, sCC, sQM, sMB, sAD, sEBa, sEBv,
         sS2, sMC, sECa, sECv) = (
            ctx.enter_context(nc.semaphore(n))
            for n in ("sFt", "sMb", "sW", "sG", "sMA", "sEA", "sS1", "sCC",
                      "sQM", "sMB", "sAD", "sEBa", "sEBv", "sS2", "sMC",
                      "sECa", "sECv")
        )
        sQh = [ctx.enter_context(nc.semaphore(f"sQ{i}")) for i in range(4)]
        sYs = ctx.enter_context(nc.semaphore("sYs"))
        sSt = ctx.enter_context(nc.semaphore("sSt"))
        block = ctx.enter_context(nc.Block())

        # views
        def qk_v(b, k):
            return qk_sb[:, ((b % 2) * KT + k) * E : ((b % 2) * KT + k + 1) * E]

        def ft_v(k):
            return ft_sb[:, 64 * k : 64 * (k + 1)]

        def w_v(t, j, ch):
            o = ((t * 3 + j) * NCH + ch) * E
            return w_sb[:, o : o + E]

        def psA_v(b, ch):
            bank = (b % 2) * 4 + ch
            return ps[:, 512 * bank : 512 * bank + 64]

        def psB_v(t, ri):
            x = 2 * (t % 2) + ri
            bank = 4 * (t // 2) + x
            return ps[32 * x : 32 * (x + 1), 512 * bank : 512 * (bank + 1)]

        def psC_v(idx):
            bank = idx % 8
            return ps[:, 512 * bank : 512 * (bank + 1)]

        qa_r = qa_sb.rearrange(
            "p (j b ch u) -> p j b ch u", j=NCORES, b=BL, ch=NCH, u=GRP
        )

        def qa_v(b, ch):
            return qa_r[:, :, b, ch, :]  # (128, 8, 8) strided

        qm_r = qm_sb.rearrange(
            "p (jb ch u) -> p ch u jb", jb=B, ch=NCH, u=GRP
        )

        def ys_v(t, ri):
            x = 2 * (t % 2) + ri
            return ys_sb[32 * x : 32 * (x + 1), (t // 2) * E : (t // 2 + 1) * E]

        def yst_v(b):
            return yst_sb[:, b * E : (b + 1) * E]

        def ob_v2(pidx):
            # pair eviction: psC banks [2*pidx, 2*pidx+1] -> ob cols
            return ob_sb[:, 2 * pidx * E : (2 * pidx + 2) * E]

        def psC_pair(pidx):
            bank = (2 * pidx) % 8
            return ps[:, 512 * bank : 512 * (bank + 2)]

        def ob_b(bb):
            return ob_sb[:, bb * 16 * E : (bb + 1) * 16 * E]

        # C evictions in PAIRS of psC banks; alternate ACT / DVE per pair
        EV_ENG = [("a", "v")[i % 2] for i in range(BL * 8)]
        EV_ENG[29] = "a"  # DVE pair-copies are ~10% slower; rebalance 17/15
        EV_SEM = {"a": sECa, "v": sECv}

        def ev_count(eng, upto_pair):
            return sum(1 for i in range(upto_pair + 1) if EV_ENG[i] == eng)

        def wait_evs(eng_obj, upto_pair):
            for e in ("a", "v"):
                n = ev_count(e, upto_pair)
                if n:
                    eng_obj.wait_ge(EV_SEM[e], n)

        # store ring per (b, q) quarter: SP does q 0,1; GP does q 2,3
        def store_dma(ring, bb, q, sem):
            wait_evs(ring, 8 * bb + 2 * q + 1)
            ring.dma_start(
                out=out_ext[bb][:, 4 * q * E : 4 * (q + 1) * E],
                in_=ob_b(bb)[:, 4 * q * E : 4 * (q + 1) * E],
            ).then_inc(sem, 16)

        # ---------------- SP ring: input streams + stores q0/q1 ----------------
        @block.sync
        def _(sync):
            for b in range(BL):
                if b >= 2:
                    sync.wait_ge(sMA, 4 * (b - 1))  # batch b-2 fully consumed
                for h in range(2):
                    sync.dma_start(
                        out=qk_sb.rearrange("p (s k e) -> p s k e", s=4, k=KT // 2)[
                            :, (b % 2) * 2 + h
                        ],
                        in_=q_ext[b].rearrange("(k p) e -> p k e", p=128)[
                            :, 8 * h : 8 * (h + 1)
                        ],
                    ).then_inc(sQh[(b % 2) * 2 + h], 16)
            # qm load (after collective 1): SP half (j 0-3)
            sync.wait_ge(sCC, 1)
            sync.dma_start(
                out=qm_sb.rearrange("p (j f) -> p j f", j=NCORES)[:, :4],
                in_=a1_out.rearrange("j (p f) -> p j f", p=128)[:, :4],
            ).then_inc(sQM, 16)
            # staging 2: SP stages x 2,3
            for x in range(2, 4):
                sync.wait_ge(sEBa, 2 * (x // 2) + (x % 2) + 1)
                sync.wait_ge(sEBv, 2 * (x // 2) + (x % 2) + 1)
                sync.dma_start(
                    out=a2_in.rearrange("j (x blc) -> x j blc", x=4, blc=BL * 2 * E)[x],
                    in_=ys_sb[32 * x : 32 * (x + 1), :],
                ).then_inc(sS2, 16)
            # yst loads (after collective 2): all issued before any stores
            sync.wait_ge(sCC, 2)
            for b in range(BL):
                for half in range(2):
                    sync.dma_start(
                        out=yst_v(b)[64 * half : 64 * (half + 1), :],
                        in_=a2_out.rearrange(
                            "j (x bl pp p) -> bl j x pp p", x=4, bl=BL, pp=2, p=E
                        )[b],
                    ).then_inc(sYs, 16)
            for b in range(BL):
                for q in range(2):
                    store_dma(sync, b, q, sSt)

        # ---------------- PE: all matmuls ----------------
        @block.tensor
        def _(pe):
            pe.wait_ge(sFt, 16)  # ft loaded
            for b in range(BL):
                if b >= 2:
                    pe.wait_ge(sEA, 4 * (b - 1))  # psum bank set evicted
                for k in range(KT):
                    if k % 8 == 0:
                        pe.wait_ge(sQh[(b % 2) * 2 + k // 8], 16 * (b // 2 + 1))
                    for ch in range(NCH):
                        mm = pe.matmul(
                            psA_v(b, ch),
                            qk_v(b, k)[:, 128 * ch : 128 * (ch + 1)],
                            ft_v(k),
                            start=(k == 0),
                            stop=(k == KT - 1),
                        )
                        if k == KT - 1:
                            mm.then_inc(sMA, 1)
            # stage B
            pe.wait_ge(sW, 64)
            pe.wait_ge(sG, 16)
            pe.wait_ge(sQM, 32)
            for ps_i_ in range(2):
                for ch in range(NCH):
                    for tl in range(2):
                        t = 2 * ps_i_ + tl
                        lhs_r = qm_r[:, ch, t, :]
                        lhs_i = qm_r[:, ch, ML + t, :]
                        first, last = ch == 0, ch == NCH - 1
                        tp0 = (0, 32 * (2 * tl + 0))
                        tp1 = (0, 32 * (2 * tl + 1))
                        pe.matmul(psB_v(t, 0), lhs_r, w_v(t, 0, ch),
                                  start=first, stop=False, tile_position=tp0)
                        pe.matmul(psB_v(t, 1), lhs_r, w_v(t, 1, ch),
                                  start=first, stop=False, tile_position=tp1)
                        m3 = pe.matmul(psB_v(t, 0), lhs_i, w_v(t, 2, ch),
                                       start=False, stop=last, tile_position=tp0)
                        m4 = pe.matmul(psB_v(t, 1), lhs_i, w_v(t, 0, ch),
                                       start=False, stop=last, tile_position=tp1)
                        if last:
                            m3.then_inc(sMB, 1)
                            m4.then_inc(sMB, 1)
            # stage C
            for b in range(BL):
                pe.wait_ge(sYs, 32 * (b + 1))
                for lch in range(0, 16, 2):
                    idx = b * 16 + lch
                    if idx >= 8:
                        wait_evs(pe, (idx - 8) // 2)
                    pe.matmul(
                        psC_v(idx),
                        g_sb[0:64, 128 * lch : 128 * (lch + 1)],
                        yst_v(b)[0:64, :],
                        start=True,
                        stop=True,
                        tile_position=(0, 0),
                    ).then_inc(sMC, 1)
                    pe.matmul(
                        psC_v(idx + 1),
                        g_sb[64:128, 128 * (lch + 1) : 128 * (lch + 2)],
                        yst_v(b)[64:128, :],
                        start=True,
                        stop=True,
                        tile_position=(64, 0),
                    ).then_inc(sMC, 1)

        # ------------- ACT ring: consts, evictions, staging, stores q2/q3 -----
        @block.scalar
        def _(act):
            act.dma_start(out=ft_sb[:], in_=ft_ext[:]).then_inc(sFt, 16)
            if with_bias:
                act.dma_start(out=mb_sb[:], in_=mb_ext[:]).then_inc(sMb, 16)
            act.dma_start(out=g_sb[:], in_=g_ext[:]).then_inc(sG, 16)
            # stage A evictions (f32 -> bf16)
            for b in range(BL):
                for ch in range(NCH):
                    act.wait_ge(sMA, 4 * b + ch + 1)
                    act.copy(
                        out=qa_v(b, ch),
                        in_=psA_v(b, ch).rearrange("p (j u) -> p j u", j=NCORES),
                    ).then_inc(sEA, 1)
            # staging 1
            act.wait_ge(sEA, 16)
            act.dma_start(
                out=a1_in.rearrange("j (p f) -> p j f", p=128),
                in_=qa_sb.rearrange("p (j f) -> p j f", j=NCORES),
            ).then_inc(sS1, 16)
            # w loads drain during collective 1
            for t in range(ML):
                act.dma_start(
                    out=w_sb[:, t * 3 * NCH * E : (t + 1) * 3 * NCH * E],
                    in_=w_ext[t],
                ).then_inc(sW, 16)
            # qm load ACT half (j 4-7)
            act.wait_ge(sCC, 1)
            act.dma_start(
                out=qm_sb.rearrange("p (j f) -> p j f", j=NCORES)[:, 4:],
                in_=a1_out.rearrange("j (p f) -> p j f", p=128)[:, 4:],
            ).then_inc(sQM, 16)
            # stage B evictions: ACT does t 0,1; DVE does t 2,3
            for t in range(2):
                for ri in range(2):
                    if t == 0 and ri == 0 and with_bias:
                        act.wait_ge(sAD, 1)
                    else:
                        act.wait_ge(sMB, 2 * t + ri + 1)
                    act.copy(out=ys_v(t, ri), in_=psB_v(t, ri)).then_inc(sEBa, 1)
            # staging 2: ACT stages x 0,1 (SP stages x 2,3)
            for x in range(2):
                act.wait_ge(sEBa, 2 * (x // 2) + (x % 2) + 1)
                act.wait_ge(sEBv, 2 * (x // 2) + (x % 2) + 1)
                act.dma_start(
                    out=a2_in.rearrange("j (x blc) -> x j blc", x=4, blc=BL * 2 * E)[x],
                    in_=ys_sb[32 * x : 32 * (x + 1), :],
                ).then_inc(sS2, 16)
            # stage C evictions (ACT share of bank pairs)
            for pidx in range(BL * 8):
                if EV_ENG[pidx] != "a":
                    continue
                act.wait_ge(sMC, 2 * pidx + 2)
                act.copy(out=ob_v2(pidx), in_=psC_pair(pidx)).then_inc(sECa, 1)

        # ------------- DVE: negation, bias add, 1/3 of C evictions -------------
        @block.vector
        def _(dve):
            if with_bias:
                dve.wait_ge(sMb, 16)  # mb loaded
                dve.wait_ge(sMB, 1)   # t=0 yr chain done
                dve.tensor_add(psB_v(0, 0), psB_v(0, 0), mb_sb[:]).then_inc(sAD, 1)
            # stage B evictions: DVE share (t 2,3)
            for t in range(2, ML):
                for ri in range(2):
                    dve.wait_ge(sMB, 4 + 2 * (t - 2) + ri + 1)
                    dve.tensor_copy(ys_v(t, ri), psB_v(t, ri)).then_inc(sEBv, 1)
            for pidx in range(BL * 8):
                if EV_ENG[pidx] != "v":
                    continue
                dve.wait_ge(sMC, 2 * pidx + 2)
                dve.tensor_copy(ob_v2(pidx), psC_pair(pidx)).then_inc(sECv, 1)

        # ---------------- GPSIMD: collectives + 1/3 of C evictions ----------------
        @block.gpsimd
        def _(gp):
            gp.wait_ge(sS1, 16)
            gp.collective_compute(
                "AllToAll",
                mybir.AluOpType.bypass,
                replica_groups=rg,
                ins=[a1_in[:]],
                outs=[a1_out[:]],
            ).then_inc(sCC, 1)
            gp.wait_ge(sS2, 56)
            gp.collective_compute(
                "AllToAll",
                mybir.AluOpType.bypass,
                replica_groups=rg,
                ins=[a2_in[:]],
                outs=[a2_out[:]],
            ).then_inc(sCC, 1)
            for bb in range(BL):
                for q in range(2, 4):
                    store_dma(gp, bb, q, sSt)

    return nc


def _host_prep(queries, Wq, bq, W1r, W1i, Wo, bo):
    """Fold Wq/Wo into per-mode weights, build DFT matrices, shard per core."""
    l = np.arange(L)
    m = np.arange(MODES)
    ang = 2.0 * np.pi * np.outer(m, l) / L          # (M, L)
    cos_ml = np.cos(ang)
    sin_ml = np.sin(ang)

    # DFT moving tiles, packed [p_in_tile, k*64 + c]; c: group g -> [cos, -sin]
    ft = np.empty((L, 64), np.float32)
    for g in range(NCORES):
        ft[:, GRP * g : GRP * g + ML] = cos_ml[4 * g : 4 * g + ML].T
        ft[:, GRP * g + ML : GRP * (g + 1)] = -sin_ml[4 * g : 4 * g + ML].T
    ft = np.ascontiguousarray(
        ft.reshape(KT, 128, 64).transpose(1, 0, 2).reshape(128, KT * 64)
    )

    # Folded mode weights: W'_m = Wq.T @ (W1r_m + i W1i_m) @ Wo.T
    Wq64 = Wq.astype(np.float64)
    Wo64 = Wo.astype(np.float64)
    Wpr = np.empty((E, E, MODES), np.float32)
    Wpi = np.empty((E, E, MODES), np.float32)
    for mm in range(MODES):
        ar = Wq64.T @ W1r[:, :, mm].astype(np.float64)
        ai = Wq64.T @ W1i[:, :, mm].astype(np.float64)
        Wpr[:, :, mm] = (ar @ Wo64.T).astype(np.float32)
        Wpi[:, :, mm] = (ai @ Wo64.T).astype(np.float32)

    # Inverse DFT rows g[k = j*8 + t*2 + ri, l]
    cm = np.where(m == 0, 1.0, 2.0)
    g_mat = np.empty((64, L), np.float32)
    for r in range(64):
        c, x, pp = r // 8, (r % 8) // 2, r % 2
        tl, ri = x // 2, x % 2
        mm = 4 * c + 2 * pp + tl
        if ri == 0:
            g_mat[r] = cm[mm] * cos_ml[mm] / L
        else:
            g_mat[r] = -cm[mm] * sin_ml[mm] / L
            if mm == 0:
                g_mat[r] = 0.0  # irfft ignores Im(bin 0)

    out_bias = (
        bo.astype(np.float64)
        + bq.astype(np.float64) @ W1r[:, :, 0].astype(np.float64) @ Wo64.T
    ).astype(np.float32)

    ft_b = ft.astype(BF16)
    g_b = np.vstack([g_mat, g_mat]).astype(BF16)

    in_maps = []
    for c in range(NCORES):
        w_pack = np.empty((ML, 128, 3, NCH, E), np.float32)
        for t in range(ML):
            mm = 4 * c + t
            for ch in range(NCH):
                w_pack[t, :, 0, ch] = Wpr[128 * ch : 128 * (ch + 1), :, mm]
                w_pack[t, :, 1, ch] = Wpi[128 * ch : 128 * (ch + 1), :, mm]
                w_pack[t, :, 2, ch] = -Wpi[128 * ch : 128 * (ch + 1), :, mm]
        w_pack = w_pack.reshape(ML, 128, 3 * NCH * E)
        in_maps.append(
            {
                "q": np.ascontiguousarray(queries[BL * c : BL * (c + 1)]).astype(BF16),
                "ft": ft_b,
                "w": w_pack.astype(BF16),
                "g": g_b,
                "mb": np.broadcast_to(
                    L * out_bias[None, :] if c == 0 else np.zeros((1, E), np.float32),
                    (B, E),
                ).astype(np.float32),
            }
        )
    return in_maps


def kernel(queries, Wq, bq, W1r, W1i, Wo, bo, _trace=False):
    global _nc_cache
    with_bias = bool(np.any(bq) or np.any(bo))
    if with_bias not in _nc_cache:
        _nc_cache[with_bias] = build_nc(with_bias)
    nc = _nc_cache[with_bias]

    in_maps = _host_prep(queries, Wq, bq, W1r, W1i, Wo, bo)
    res = run_bass_kernel_spmd(nc, in_maps, core_ids=list(range(NCORES)), trace=_trace)
    results = res.results
    out = np.concatenate(
        [
            np.asarray(r["out"])
            .reshape(BL, 128, KT, E)
            .transpose(0, 2, 1, 3)
            .reshape(BL, L, E)
            for r in results
        ],
        axis=0,
    )
    if _trace:
        kernel._last = res
    return out.astype(np.float32)

